# revision 1
# baseline (speedup 1.0000x reference)
"""Trainium2 kernel for nn_CompressedAttention: 8-core SPMD, head-sharded attention.

Sharding: core c owns heads {2c, 2c+1} for both batches. Host decompresses the
rFFT-compressed weights (layout + FFT prep), device runs all GEMMs + softmax:
  qkv^T GEMMs -> S^T = K^T.T@Q^T blocks -> exp (no max-sub; scores are O(1)) ->
  PV with ones-column row-sums -> normalize -> transpose-DMA to O^T ->
  per-core partial projection (its 128 Wp rows). Host sums partials + bias.
"""
import sys
import numpy as np

sys.path.insert(0, '/opt/trn_rl_repo')

import concourse.bass as bass
import concourse.mybir as mybir
from concourse import tile
from concourse.bass_utils import run_bass_kernel_spmd
import ml_dtypes

N_CORES = 8
D = 1024
NH = 16
HD = 64
B, S = 2, 2048
T = B * S  # 4096
BF = mybir.dt.bfloat16
F32 = mybir.dt.float32

# ---------------- tile exit-barrier compile fixes (walrus sync-wait limits) ----
import json as _json
import concourse.tile as _tile_mod


def _patched_dab(self, tick_clock, wait_clock):
    nc = self.nc
    drain_inst = nc.sync.drain()
    wait_clock.add_sem_waits(drain_inst.ins, _tile_mod.ScopedClock({None: tick_clock.global_clock}))
    bar = nc.alloc_semaphore("final_bar")
    for eng in nc.engines.values():
        eng.nop().then_inc(bar, 1)
    for eng in nc.engines.values():
        eng.wait_ge(bar, len(nc.engines))
    popped = nc._tile_sem_poison_stack.pop()
    assert popped is self._sem_poison
    nc.clear_and_free_semaphores(list(self.sems.allocated().values()) + [bar])


_tile_mod.TileContext._drain_and_barrier = _patched_dab


def _split_wide_waits(j, max_waits=1):
    for fn in j['functions']:
        for bb in fn['blocks']:
            out = []
            for ins in bb['instructions']:
                si = ins.get('sync_info')
                ow = (si or {}).get('on_wait') or []
                if len(ow) > max_waits:
                    chunks = [ow[i:i + max_waits] for i in range(0, len(ow), max_waits)]
                    for ci, ch in enumerate(chunks[:-1]):
                        out.append({'debug': ins.get('debug', 0), 'engine': ins['engine'],
                                    'ins': [], 'outs': [], 'name': ins['name'] + f'_w{ci}',
                                    'opcode': 'NoOp',
                                    'sync_info': {'on_update': [], 'on_wait': ch}})
                    si['on_wait'] = chunks[-1]
                out.append(ins)
            bb['instructions'] = out
    return j


def _patch_json(nc):
    orig = nc.to_json_bytes

    def patched():
        return _json.dumps(_split_wide_waits(_json.loads(orig()))).encode()

    nc.to_json_bytes = patched


# ---------------- host decompress (irfft of scattered top-k spectrum) ----------
def _decompress(re, im, idx, fft_len, pad_n, n, shape, scale):
    full = np.zeros(fft_len, np.complex64)
    full[idx] = re + 1j * im
    return (np.fft.irfft(full, n=pad_n)[:n].reshape(shape) * scale[0]).astype(np.float32)


# ---------------- device kernel -----------------------------------------------
def build_kernel():
    nc = bass.Bass()
    xt = nc.declare_dram_parameter("xt", [D, T], BF, isOutput=False)          # x^T bf16
    waq = nc.declare_dram_parameter("waq", [D, 128], BF, isOutput=False)      # Wa q-cols of pair
    wak = nc.declare_dram_parameter("wak", [D, 128], BF, isOutput=False)
    wav = nc.declare_dram_parameter("wav", [D, 132], BF, isOutput=False)      # padded v layout
    bq = nc.declare_dram_parameter("bq", [128, 1], F32, isOutput=False)
    bk = nc.declare_dram_parameter("bk", [128, 1], F32, isOutput=False)
    bvrow = nc.declare_dram_parameter("bvrow", [1, 132], BF, isOutput=False)  # [bv0|1|0|bv1|1|0]
    onesk = nc.declare_dram_parameter("onesk", [1, 128], BF, isOutput=False)
    wp = nc.declare_dram_parameter("wp", [128, D], BF, isOutput=False)        # Wp rows of pair
    out = nc.declare_dram_parameter("out_part", [T, D], F32, isOutput=True)

    with tile.TileContext(nc) as tc:
        with tc.tile_pool(name="const", bufs=1) as cpool:
            # resident tensors
            xt_sb = [cpool.tile([128, T], BF, tag=f"xt{i}", name=f"xt{i}") for i in range(8)]
            for i in range(8):
                nc.sync.dma_start(out=xt_sb[i][:], in_=xt[128 * i:128 * (i + 1), :])
            waq_sb = cpool.tile([128, 8 * 128], BF, tag="waq", name="waq_t")
            wak_sb = cpool.tile([128, 8 * 128], BF, tag="wak", name="wak_t")
            wav_sb = cpool.tile([128, 8 * 132], BF, tag="wav", name="wav_t")
            for i in range(8):
                nc.sync.dma_start(out=waq_sb[:, 128 * i:128 * (i + 1)], in_=waq[128 * i:128 * (i + 1), :])
                nc.sync.dma_start(out=wak_sb[:, 128 * i:128 * (i + 1)], in_=wak[128 * i:128 * (i + 1), :])
                nc.sync.dma_start(out=wav_sb[:, 132 * i:132 * (i + 1)], in_=wav[128 * i:128 * (i + 1), :])
            bq_sb = cpool.tile([128, 1], F32, tag="bq", name="bq_t")
            bk_sb = cpool.tile([128, 1], F32, tag="bk", name="bk_t")
            nc.sync.dma_start(out=bq_sb[:], in_=bq[:])
            nc.sync.dma_start(out=bk_sb[:], in_=bk[:])
            bvrow_sb = cpool.tile([1, 132], BF, tag="bvrow", name="bvrow_t")
            onesk_sb = cpool.tile([1, 128], BF, tag="onesk", name="onesk_t")
            nc.sync.dma_start(out=bvrow_sb[:], in_=bvrow[:])
            nc.sync.dma_start(out=onesk_sb[:], in_=onesk[:])
            wp_sb = cpool.tile([128, D], BF, tag="wp", name="wp_t")
            nc.sync.dma_start(out=wp_sb[:], in_=wp[:])
            qt_sb = cpool.tile([128, T], BF, tag="qt", name="qt_t")   # Q^T for pair [128 f, 4096]
            kt_sb = cpool.tile([128, T], BF, tag="kt", name="kt_t")
            v_sb = cpool.tile([128, 32 * 132], BF, tag="v", name="v_t")  # V tiles per 128-tok chunk
            ot_sb = cpool.tile([128, T], BF, tag="ot", name="ot_t")   # O^T accum [128 dpair, 4096]

            # ---- phase B: Q^T, K^T ----
            with tc.tile_pool(name="qk_ps", bufs=4, space="PSUM") as qkps:
                for (w_sb, b_sb, dst) in ((waq_sb, bq_sb, qt_sb), (wak_sb, bk_sb, kt_sb)):
                    for tci in range(8):  # 512-wide token chunks
                        ps = qkps.tile([128, 512], F32, tag="qkps", name="qkps_t")
                        for dc in range(8):
                            nc.tensor.matmul(ps[:],
                                             lhsT=w_sb[:, 128 * dc:128 * (dc + 1)],
                                             rhs=xt_sb[dc][:, 512 * tci:512 * (tci + 1)],
                                             start=(dc == 0), stop=(dc == 7))
                        nc.vector.tensor_scalar_add(dst[:, 512 * tci:512 * (tci + 1)], ps[:], b_sb[:])

            # ---- phase C: V (+ ones column) ----
            with tc.tile_pool(name="v_ps", bufs=4, space="PSUM") as vps:
                for vtc in range(32):  # 128-tok chunks
                    ps = vps.tile([128, 132], F32, tag="vps", name="vps_t")
                    nc.tensor.matmul(ps[:], lhsT=onesk_sb[:], rhs=bvrow_sb[:], start=True, stop=False)
                    for dc in range(8):
                        nc.tensor.matmul(ps[:, 0:64],
                                         lhsT=xt_sb[dc][:, 128 * vtc:128 * (vtc + 1)],
                                         rhs=wav_sb[:, 132 * dc:132 * dc + 64],
                                         start=False, stop=False)
                        nc.tensor.matmul(ps[:, 66:130],
                                         lhsT=xt_sb[dc][:, 128 * vtc:128 * (vtc + 1)],
                                         rhs=wav_sb[:, 132 * dc + 66:132 * dc + 130],
                                         start=False, stop=(dc == 7))
                    nc.scalar.activation(v_sb[:, 132 * vtc:132 * (vtc + 1)], ps[:],
                                         mybir.ActivationFunctionType.Copy)

            # ---- phase D: attention ----
            with tc.tile_pool(name="s_ps", bufs=4, space="PSUM") as sps, \
                 tc.tile_pool(name="pv_ps", bufs=4, space="PSUM") as pvps, \
                 tc.tile_pool(name="p_sb", bufs=18) as ppool, \
                 tc.tile_pool(name="o_sb", bufs=4) as opool, \
                 tc.tile_pool(name="r_sb", bufs=8) as rpool:
                for b in range(B):
                    for qc in range(4):  # 512-wide query chunks within batch
                        q0 = 2048 * b + 512 * qc
                        ostage = [opool.tile([128, 128], BF, tag="ost", name="ost_t") for _ in range(4)]
                        for h in range(2):
                            hr0 = 64 * h
                            ptiles = []
                            for kb in range(16):  # 128-wide key blocks
                                k0 = 2048 * b + 128 * kb
                                ps = sps.tile([128, 512], F32, tag="sps", name="sps_t")
                                nc.tensor.matmul(ps[:],
                                                 lhsT=kt_sb[hr0:hr0 + 64, k0:k0 + 128],
                                                 rhs=qt_sb[hr0:hr0 + 64, q0:q0 + 512],
                                                 start=True, stop=True)
                                pt = ppool.tile([128, 512], BF, tag="pt", name="pt_t")
                                nc.scalar.activation(pt[:], ps[:],
                                                     mybir.ActivationFunctionType.Exp,
                                                     scale=0.125)
                                ptiles.append(pt)
                            for qs in range(4):  # 128-wide query sub-chunks
                                pv = pvps.tile([128, 65], F32, tag="pvps", name="pvps_t")
                                vtc0 = 16 * b
                                for kb in range(16):
                                    nc.tensor.matmul(
                                        pv[:],
                                        lhsT=ptiles[kb][:, 128 * qs:128 * (qs + 1)],
                                        rhs=v_sb[:, 132 * (vtc0 + kb) + 66 * h:132 * (vtc0 + kb) + 66 * h + 65],
                                        start=(kb == 0), stop=(kb == 15))
                                rec = rpool.tile([128, 1], F32, tag="rec", name="rec_t")
                                nc.vector.reciprocal(rec[:], pv[:, 64:65])
                                nc.vector.tensor_scalar_mul(
                                    ostage[qs][:, 64 * h:64 * (h + 1)], pv[:, 0:64], rec[:])
                        for qs in range(4):
                            nc.sync.dma_start(out=ot_sb[:, q0 + 128 * qs:q0 + 128 * (qs + 1)],
                                              in_=ostage[qs][:], transpose=True)

            # ---- phase E: partial projection ----
            with tc.tile_pool(name="pr_ps", bufs=4, space="PSUM") as prps, \
                 tc.tile_pool(name="pr_sb", bufs=4) as prsb:
                for tci in range(32):  # 128-tok chunks
                    for ec in range(2):  # 512-wide output cols
                        ps = prps.tile([128, 512], F32, tag="prps", name="prps_t")
                        nc.tensor.matmul(ps[:],
                                         lhsT=ot_sb[:, 128 * tci:128 * (tci + 1)],
                                         rhs=wp_sb[:, 512 * ec:512 * (ec + 1)],
                                         start=True, stop=True)
                        os_ = prsb.tile([128, 512], F32, tag="prsb", name="prsb_t")
                        nc.scalar.activation(os_[:], ps[:], mybir.ActivationFunctionType.Copy)
                        nc.sync.dma_start(out=out[128 * tci:128 * (tci + 1), 512 * ec:512 * (ec + 1)],
                                          in_=os_[:])
    _patch_json(nc)
    return nc


_NC_CACHE = None
_LAST_IN_MAPS = None
_LAST_RUN_S = None


def kernel(**inputs) -> np.ndarray:
    global _NC_CACHE
    x = np.asarray(inputs['x'])
    Wa = _decompress(np.asarray(inputs['c_attn_re']), np.asarray(inputs['c_attn_im']),
                     np.asarray(inputs['c_attn_idx']), 2097153, 4194304, 3145728,
                     (1024, 3072), np.asarray(inputs['c_attn_scale']))
    Wp = _decompress(np.asarray(inputs['c_proj_re']), np.asarray(inputs['c_proj_im']),
                     np.asarray(inputs['c_proj_idx']), 524289, 1048576, 1048576,
                     (1024, 1024), np.asarray(inputs['c_proj_scale']))
    ca_b = np.asarray(inputs['c_attn_bias'])
    cp_b = np.asarray(inputs['c_proj_bias'])

    bf = ml_dtypes.bfloat16
    xt = np.ascontiguousarray(x.reshape(T, D).T).astype(bf)
    in_maps = []
    for c in range(N_CORES):
        j0 = 128 * c
        wav_p = np.zeros((D, 132), np.float32)
        wav_p[:, 0:64] = Wa[:, 2048 + j0:2048 + j0 + 64]
        wav_p[:, 66:130] = Wa[:, 2048 + j0 + 64:2048 + j0 + 128]
        bvrow = np.zeros((1, 132), np.float32)
        bvrow[0, 0:64] = ca_b[2048 + j0:2048 + j0 + 64]
        bvrow[0, 64] = 1.0
        bvrow[0, 66:130] = ca_b[2048 + j0 + 64:2048 + j0 + 128]
        bvrow[0, 130] = 1.0
        in_maps.append(dict(
            xt=xt,
            waq=np.ascontiguousarray(Wa[:, j0:j0 + 128]).astype(bf),
            wak=np.ascontiguousarray(Wa[:, 1024 + j0:1024 + j0 + 128]).astype(bf),
            wav=wav_p.astype(bf),
            bq=np.ascontiguousarray(ca_b[j0:j0 + 128].reshape(128, 1)),
            bk=np.ascontiguousarray(ca_b[1024 + j0:1024 + j0 + 128].reshape(128, 1)),
            bvrow=bvrow.astype(bf),
            onesk=np.ones((1, 128), bf),
            wp=np.ascontiguousarray(Wp[j0:j0 + 128, :]).astype(bf),
        ))

    global _LAST_IN_MAPS
    _LAST_IN_MAPS = in_maps
    if _NC_CACHE is None:
        _NC_CACHE = build_kernel()
    import time as _time
    _t0 = _time.time()
    res = run_bass_kernel_spmd(_NC_CACHE, in_maps, core_ids=list(range(N_CORES)))
    global _LAST_RUN_S
    _LAST_RUN_S = _time.time() - _t0
    acc = np.zeros((T, D), np.float64)
    for c in range(N_CORES):
        acc += res.results[c]['out_part'].astype(np.float64)
    acc += cp_b[None, :]
    return acc.astype(np.float32).reshape(B, S, D)

